# revision 21
# baseline (speedup 1.0000x reference)
"""BitNet MLP (nn_BitNetMLP_19421842112750) — TRN2 Bass kernel, 8-core
data-parallel over tokens.  v2: startup/transition/drain overhead removed.

Per core (T=1024 tokens of the 8192 total):
  G = x @ Wg_tern.T ; U = x @ Wu_tern.T          (ternary weights, scales folded)
  r = relu(G)^2 * U                               (= hidden_ref / c, c = gs^2*us)
  hn = r * norm_w ; acc += r^2
  var' = (ones @ acc) / I + eps/c^2               (= (var_ref + eps)/c^2, bcast 128p)
  out = (hn @ Wd_tern.T) * sqrt(ds^2 / var')      (== reference output exactly)

v2 changes vs v1 (1.551 ms):
  - x is pre-cast to bf16 on host and DMA'd straight into the persistent
    xT tile: no f32 staging, no on-device casts (v1 lost 71 us at startup
    because 20 Vector-engine casts preceded the band-0 weight shifts in
    queue order).
  - variance is accumulated on Scalar(square)+Vector(add) engines into an
    SBUF f32 acc tile; one f32 ones-matmul reduces+broadcasts it across
    partitions (v1 spent ~23 us of TensorE on 108 FD-512 variance matmuls
    and serialized phase B behind a PSUM pool handoff).
  - gate/up unpack is widened: one [128, 640B] stage DMA + 4 shifts + 1
    ACT copy per matrix per band (v1: 5 DMAs + 20 shifts + 5 copies) with
    a k-major chunk order absorbed into perm_H.
  - the ragged 256-row down-proj tail is merged into 2 full-K=128 matmuls
    per (ht,t) via half-partition shifts (v1: 4 half-K matmuls that cost
    full FD cycles each; -80 matmuls).
  - down-proj epilogue (rstd mul + store) is emitted per-(ht,t) right
    after that tile's closing matmul, so the final drain is one tile deep
    (v1: 18 us tail), and HBAND=3 + single-buffered PSUM lets ps_var (2
    banks) coexist with the 6 po banks.

Device layouts:
  xTp   [H, T]   bf16 rows: h = pi_H(r)        (per-core x shard, transposed)
  gpTp  [H/4, I] u8   cols: i = pi_I(c)        (same for upTp)
  dpT   [I/4, H] u8   natural (= down_packed.T)
  nw_p  [I]      f32  nw_p[r] = norm_w[pi_I(r)]
  outT  [H, T]   f32  natural h rows (host transposes back)
"""

import sys

sys.path.insert(0, "/opt/trn_rl_repo")
from contextlib import ExitStack

import numpy as np
import ml_dtypes

import concourse.bass as bass
import concourse.tile as tile
from concourse import bacc, mybir

F32 = mybir.dt.float32
BF16 = mybir.dt.bfloat16
U8 = mybir.dt.uint8
U32 = mybir.dt.uint32
SWAR_MASK = 0x03030303
AOT = mybir.AluOpType
ACTF = mybir.ActivationFunctionType
RMS_EPS = 1e-6

N_CORES = 8
FULL_B, FULL_S, FULL_H, FULL_I = 4, 2048, 2560, 6912


# ---------------------------------------------------------------- permutations
def perm_H(n, G=5):
    """SBUF row r = 128*c + p -> original h index, k-major chunk order:
    chunk c holds shift-plane k = c//G of DMA row-group g = c%G."""
    assert n == 512 * G
    r = np.arange(n)
    c, p = r // 128, r % 128
    return 512 * (c % G) + 128 * (p // 32) + 32 * (c // G) + (p % 32)


def perm_I(n):
    """hidden SBUF row r -> original i index. Full 512-groups, then a
    256-tail (two 128-tiles, each split into 64-partition halves)."""
    r = np.arange(n)
    c, p = r // 128, r % 128
    out = 512 * (c // 4) + 128 * (p // 32) + 32 * (c % 4) + (p % 32)
    n_full = (n // 512) * 512
    if n_full != n:
        assert n - n_full == 256, "tail must be exactly 256"
        off = r[n_full:] - n_full
        tile_off, p2 = off // 128, off % 128
        s, q, j = p2 // 64, (p2 % 64) // 32, p2 % 32
        k = 2 * tile_off + s
        out[n_full:] = n_full + 128 * q + 32 * k + j
    return out


# ---------------------------------------------------------------- the program
def build_program(T, H, I, W_I=128, HB=3):
    """Build the single-core Bass program (SPMD-identical across cores)."""
    TC = 512
    G = H // 512  # 5 DMA row-groups per gate/up band
    NH = H // 128  # 20 contraction chunks
    NI = I // 128  # 54 hidden i-tiles
    NT = T // TC  # 2 t-chunks
    NB = I // W_I  # 54 gate/up bands
    IB = I // 4  # 1728 down packed rows
    NC_FULL = IB // 128  # 13 full down C-groups
    C_TAIL = IB % 128  # 64
    NHB = (NH + HB - 1) // HB  # 7 down h-bands
    assert W_I == 128 and T % TC == 0 and H == 512 * G and C_TAIL == 64

    nc = bacc.Bacc("TRN2", target_bir_lowering=False, debug=False)

    xTp = nc.dram_tensor("xTp", [H, T], BF16, kind="ExternalInput").ap()
    gpTp = nc.dram_tensor("gpTp", [H // 4, I], U8, kind="ExternalInput").ap()
    upTp = nc.dram_tensor("upTp", [H // 4, I], U8, kind="ExternalInput").ap()
    dpT = nc.dram_tensor("dpT", [IB, H], U8, kind="ExternalInput").ap()
    nw = nc.dram_tensor("nw_p", [I], F32, kind="ExternalInput").ap()
    eps_in = nc.dram_tensor("eps_in", [128], F32, kind="ExternalInput").ap()
    ds2_in = nc.dram_tensor("ds2_in", [128], F32, kind="ExternalInput").ap()
    # band 0 of gate/up pre-unpacked on host: first matmul needs no on-device
    # unpack (the staging DMA's 640 small descriptors alone cost ~5 us of
    # Sync-queue issue time at startup)
    wg0_in = nc.dram_tensor("wg0", [128, NH * W_I], BF16, kind="ExternalInput").ap()
    wu0_in = nc.dram_tensor("wu0", [128, NH * W_I], BF16, kind="ExternalInput").ap()
    outT = nc.dram_tensor("outT", [H, T], F32, kind="ExternalOutput").ap()

    with tile.TileContext(nc) as tc, ExitStack() as top:
        const = top.enter_context(tc.tile_pool(name="const", bufs=1))
        hn_pool = top.enter_context(tc.tile_pool(name="hn", bufs=1))
        hn_sb = hn_pool.tile([128, NI, T], BF16)
        # phase-B staging pools are created BEFORE the phase-A pools so their
        # SBUF does not recycle A's band buffers: address reuse would add WAR
        # edges making the down-weight prefetch wait for all of phase A
        dstage = top.enter_context(tc.tile_pool(name="dstage", bufs=2))
        dsh = top.enter_context(tc.tile_pool(name="dsh", bufs=2))
        wdp = top.enter_context(tc.tile_pool(name="wdp", bufs=2))
        dtailp = top.enter_context(tc.tile_pool(name="dtailp", bufs=1))
        outp = top.enter_context(tc.tile_pool(name="outp", bufs=2))

        eps_t = const.tile([128, 1], F32)
        ds2_t = const.tile([128, 1], F32)
        nw_sb = const.tile([128, NI], F32)
        ones_f = const.tile([128, 128], F32)
        nc.gpsimd.memset(ones_f[:], 1.0)
        acc = const.tile([128, T], F32)
        nc.gpsimd.memset(acc[:], 0.0)
        rstd_bc = const.tile([128, T], F32)
        dmy = const.tile([128, 128], BF16)
        nc.gpsimd.memset(dmy[:], 0.0)

        # ================= phase A: gate/up matmuls -> hn tiles ==============
        with tc.tile_pool(name="xT", bufs=1) as xT_pool:
            xT = xT_pool.tile([128, NH, T], BF16)
            with (
                tc.tile_pool(name="wband", bufs=2) as wband,
                tc.tile_pool(name="ustage", bufs=2) as ustage,
                tc.tile_pool(name="ush", bufs=2) as ush,
                tc.tile_pool(name="psA", bufs=2, space="PSUM") as psA,
                tc.tile_pool(name="ract", bufs=2) as ract,
                tc.tile_pool(name="sqp", bufs=2) as sqp,
            ):

                def stage_band(band):
                    """DMA + unpack one gate/up band into bf16 weight tiles."""
                    i0 = band * W_I
                    wg_band = wband.tile([128, 4, G, W_I], BF16, tag="wg", name="wg")
                    wu_band = wband.tile([128, 4, G, W_I], BF16, tag="wu", name="wu")
                    for mat_ap, wt in ((gpTp, wg_band), (upTp, wu_band)):
                        st = ustage.tile([128, G, W_I], U8, tag="st", name="st")
                        nc.sync.dma_start(
                            st[:],
                            mat_ap.rearrange("(g p) i -> p g i", p=128)[
                                :, :, i0 : i0 + W_I
                            ],
                        )
                        sh = ush.tile([128, 4, G, W_I], U8, tag="sh", name="sh")
                        stw = st[:].bitcast(U32)
                        for k in range(4):
                            nc.vector.tensor_scalar(
                                sh[:, k].bitcast(U32),
                                stw,
                                6 - 2 * k,
                                SWAR_MASK,
                                AOT.logical_shift_right,
                                AOT.bitwise_and,
                            )
                        # contraction chunk c = k*G + g lives at wt[:, k, g, :]
                        nc.scalar.activation(wt[:], sh[:], ACTF.Copy, bias=-1.0)
                    return wg_band, wu_band

                # x first (its group-0 transfer bounds the first matmul), in
                # graduated groups so chunk 0 lands early; band-0 weights
                # arrive pre-unpacked (2 fast contiguous DMAs); small consts
                # go last — nothing waits on them early
                xTp_r = xTp.rearrange("(c p) t -> p c t", p=128)
                for lo, hi in ((0, 2), (2, 4), (4, 8), (8, 12), (12, 16), (16, 20)):
                    nc.sync.dma_start(xT[:, lo:hi, :], xTp_r[:, lo:hi, :])
                wg_0 = wband.tile([128, 4, G, W_I], BF16, tag="wg", name="wg")
                wu_0 = wband.tile([128, 4, G, W_I], BF16, tag="wu", name="wu")
                nc.sync.dma_start(
                    wg_0[:], wg0_in.rearrange("p (k g i) -> p k g i", k=4, g=G)
                )
                nc.sync.dma_start(
                    wu_0[:], wu0_in.rearrange("p (k g i) -> p k g i", k=4, g=G)
                )
                staged = (wg_0, wu_0)
                nc.sync.dma_start(eps_t[:], eps_in[:, None])
                nc.sync.dma_start(ds2_t[:], ds2_in[:, None])
                nc.sync.dma_start(nw_sb[:], nw.rearrange("(o p) -> p o", p=128))
                for band in range(NB):
                    wg_band, wu_band = staged
                    if band + 1 < NB:
                        staged = stage_band(band + 1)
                    pg = psA.tile([128, T], F32, tag="pg")
                    pu = psA.tile([128, T], F32, tag="pu")
                    if band == 0:
                        # warmup: keep the PE busy while x group 0 is in
                        # flight so the HAM clock-gate ramps to 8/8 before
                        # real work (outputs are discarded by the start=True
                        # of the first real accumulation group)
                        for _ in range(96):
                            nc.tensor.matmul(
                                pg[:, 0:64], dmy[:, :], dmy[:, 0:64],
                                start=True, stop=True,
                            )
                    for wt, ps in ((wg_band, pg), (wu_band, pu)):
                        for c in range(NH):
                            for t in range(NT):
                                tsl = slice(TC * t, TC * (t + 1))
                                nc.tensor.matmul(
                                    ps[:, tsl],
                                    wt[:, c // G, c % G, :],
                                    xT[:, c, tsl],
                                    start=(c == 0),
                                    stop=(c == NH - 1),
                                )
                    for t in range(NT):
                        tsl = slice(TC * t, TC * (t + 1))
                        r = ract.tile([128, TC], BF16, tag="r")
                        nc.scalar.activation(r[:], pg[:, tsl], ACTF.Relu)
                        nc.vector.tensor_mul(r[:], r[:], r[:])
                        nc.vector.tensor_mul(r[:], r[:], pu[:, tsl])
                        sq = sqp.tile([128, TC], BF16, tag="sq")
                        nc.scalar.activation(sq[:], r[:], ACTF.Square)
                        nc.vector.tensor_scalar(
                            hn_sb[:, band, tsl], r[:], nw_sb[:, band : band + 1],
                            None, AOT.mult,
                        )
                        nc.vector.tensor_add(acc[:, tsl], acc[:, tsl], sq[:])

        # ====== phase A2 + B: variance reduce/bcast overlapped with down =====
        with (
            tc.tile_pool(name="psV", bufs=1, space="PSUM") as psV,
            tc.tile_pool(name="psB", bufs=1, space="PSUM") as psB,
        ):
            ps_var = psV.tile([128, T], F32)

            def emit_variance(step):
                # variance chain sliced across the first down C-groups: the
                # ones-matmul reduces acc over partitions AND broadcasts to
                # all 128 output partitions; the 2x ~3.3 us reciprocal halves
                # are spread out so they don't head-of-line-block the Vector
                # queue in front of the down-weight shifts
                if step == 0:
                    for t in range(NT):
                        tsl = slice(TC * t, TC * (t + 1))
                        nc.tensor.matmul(
                            ps_var[:, tsl], ones_f[:], acc[:, tsl],
                            start=True, stop=True,
                        )
                    nc.scalar.activation(
                        acc[:], ps_var[:], ACTF.Identity, bias=eps_t[:], scale=1.0 / I
                    )
                elif step in (1, 2):
                    tsl = slice(TC * (step - 1), TC * step)
                    nc.vector.reciprocal(acc[:, tsl], acc[:, tsl])
                elif step == 3:
                    nc.scalar.activation(rstd_bc[:], acc[:], ACTF.Sqrt, scale=ds2_t[:])

            for hb in range(NHB):
                hbs = min(HB, NH - HB * hb)
                h0 = 128 * HB * hb
                W = 128 * hbs
                po = [
                    [
                        psB.tile([128, TC], F32, tag=f"po_{ht}_{t}", name=f"po_{ht}_{t}")
                        for t in range(NT)
                    ]
                    for ht in range(hbs)
                ]
                for C in range(NC_FULL):
                    st = dstage.tile([128, W], U8, tag="dst")
                    nc.sync.dma_start(
                        st[:], dpT[128 * C : 128 * (C + 1), h0 : h0 + W]
                    )
                    sh4 = dsh.tile([128, 4, W], U8, tag="dsh")
                    stw = st[:].bitcast(U32)
                    for k in range(4):
                        nc.vector.tensor_scalar(
                            sh4[:, k].bitcast(U32),
                            stw,
                            6 - 2 * k,
                            SWAR_MASK,
                            AOT.logical_shift_right,
                            AOT.bitwise_and,
                        )
                    wdt = wdp.tile([128, 4, W], BF16, tag="wd")
                    nc.scalar.activation(wdt[:], sh4[:], ACTF.Copy, bias=-1.0)
                    for k in range(4):
                        for ht in range(hbs):
                            hsl = slice(128 * ht, 128 * (ht + 1))
                            for t in range(NT):
                                tsl = slice(TC * t, TC * (t + 1))
                                nc.tensor.matmul(
                                    po[ht][t][:, :],
                                    wdt[:, k, hsl],
                                    hn_sb[:, 4 * C + k, tsl],
                                    start=(C == 0 and k == 0),
                                    stop=False,
                                )
                    if hb == 0 and C < 4:
                        emit_variance(C)
                # merged 256-row tail: the 64 dpT tail rows are replicated
                # into both partition halves; half-partition shifts build
                # plane-pairs (k0,k1) / (k2,k3) matching hn tiles NI-2/NI-1
                stt = dtailp.tile([128, W], U8, tag="dtail", name="stt")
                src = dpT[IB - C_TAIL : IB, h0 : h0 + W]
                nc.sync.dma_start(stt[:C_TAIL], src)
                nc.sync.dma_start(stt[C_TAIL:], src)
                sht = dtailp.tile([128, 2, W], U8, tag="dshT", name="sht")
                sttw = stt[:].bitcast(U32)
                for j in range(2):
                    nc.vector.tensor_scalar(
                        sht[:C_TAIL, j].bitcast(U32), sttw[:C_TAIL],
                        6 - 4 * j, SWAR_MASK,
                        AOT.logical_shift_right, AOT.bitwise_and,
                    )
                    nc.vector.tensor_scalar(
                        sht[C_TAIL:, j].bitcast(U32), sttw[C_TAIL:],
                        4 - 4 * j, SWAR_MASK,
                        AOT.logical_shift_right, AOT.bitwise_and,
                    )
                wtl = dtailp.tile([128, 2, W], BF16, tag="wdT", name="wtl")
                nc.scalar.activation(wtl[:], sht[:], ACTF.Copy, bias=-1.0)
                # close each po tile and emit its epilogue immediately
                for ht in range(hbs):
                    hsl = slice(128 * ht, 128 * (ht + 1))
                    for t in range(NT):
                        tsl = slice(TC * t, TC * (t + 1))
                        nc.tensor.matmul(
                            po[ht][t][:, :], wtl[:, 0, hsl],
                            hn_sb[:, NI - 2, tsl], start=False, stop=False,
                        )
                        nc.tensor.matmul(
                            po[ht][t][:, :], wtl[:, 1, hsl],
                            hn_sb[:, NI - 1, tsl], start=False, stop=True,
                        )
                        ot = outp.tile([128, TC], F32, tag="ot")
                        nc.vector.tensor_mul(ot[:], po[ht][t][:], rstd_bc[:, tsl])
                        nc.sync.dma_start(
                            outT[h0 + 128 * ht : h0 + 128 * (ht + 1), tsl], ot[:]
                        )

    nc.compile()
    return nc


# ------------------------------------------------------------- host-side prep
def _unpack_rows(packed, K):
    """SIMD block-interleaved 2-bit rows -> ternary float rows [M, K]."""
    M = packed.shape[0]
    b = packed.astype(np.int16).reshape(M, K // 128, 32)
    w = np.stack([(b >> 6) & 3, (b >> 4) & 3, (b >> 2) & 3, b & 3], axis=2)
    return w.reshape(M, K).astype(np.float32) - 1.0


def _band0_device_layout(packed, piH, piI, H):
    """Pre-unpacked band-0 weights in the device tile layout [128, NH*128]."""
    w = _unpack_rows(packed[piI[:128]], H)[:, piH]  # [128 ji, H r]
    return np.ascontiguousarray(
        w.T.reshape(H // 128, 128, 128).transpose(1, 0, 2).reshape(128, H)
    ).astype(ml_dtypes.bfloat16)


def prep_inputs(x, gate_packed, gate_scale, up_packed, up_scale, down_packed,
                down_scale, norm_w, n_cores):
    """Full inputs -> per-core in_maps in device layout (slicing + relayout)."""
    B, S, H = x.shape
    I = norm_w.shape[0]
    T_full = B * S
    T = T_full // n_cores
    piH = perm_H(H)
    piI = perm_I(I)

    gpTp = np.ascontiguousarray(gate_packed[piI].T)  # [H/4, I]
    upTp = np.ascontiguousarray(up_packed[piI].T)
    dpT = np.ascontiguousarray(down_packed.T)  # [I/4, H]
    nw_p = np.ascontiguousarray(norm_w[piI]).astype(np.float32)
    xf = x.reshape(T_full, H)

    gs_v = float(np.asarray(gate_scale).reshape(-1)[0])
    us_v = float(np.asarray(up_scale).reshape(-1)[0])
    ds_v = float(np.asarray(down_scale).reshape(-1)[0])
    c = gs_v * gs_v * us_v
    eps_p = np.full([128], RMS_EPS / (c * c), np.float32)
    ds2 = np.full([128], ds_v * ds_v, np.float32)
    wg0 = _band0_device_layout(gate_packed, piH, piI, H)
    wu0 = _band0_device_layout(up_packed, piH, piI, H)

    in_maps = []
    for cid in range(n_cores):
        xs = xf[cid * T : (cid + 1) * T]
        xTp = np.ascontiguousarray(xs.T[piH]).astype(ml_dtypes.bfloat16)  # [H, T]
        in_maps.append(
            {
                "xTp": xTp,
                "gpTp": gpTp,
                "upTp": upTp,
                "dpT": dpT,
                "nw_p": nw_p,
                "eps_in": eps_p,
                "ds2_in": ds2,
                "wg0": wg0,
                "wu0": wu0,
            }
        )
    return in_maps


def assemble_output(results, B, S, H):
    """Per-core outT [H, T] -> full [B, S, H]."""
    outs = [np.asarray(r["outT"]).T for r in results]  # each [T, H]
    return np.ascontiguousarray(np.concatenate(outs, axis=0).reshape(B, S, H))


# ---------------------------------------------------------------- entry point
_CACHED = {}


def _get_program():
    if "nc" not in _CACHED:
        T = FULL_B * FULL_S // N_CORES
        _CACHED["nc"] = build_program(T, FULL_H, FULL_I)
    return _CACHED["nc"]


def kernel(x, gate_packed, gate_scale, up_packed, up_scale, down_packed,
           down_scale, norm_w, _trace=False):
    from concourse.bass_utils import run_bass_kernel_spmd

    x = np.asarray(x, np.float32)
    gate_packed = np.asarray(gate_packed, np.uint8)
    up_packed = np.asarray(up_packed, np.uint8)
    down_packed = np.asarray(down_packed, np.uint8)
    norm_w = np.asarray(norm_w, np.float32)

    B, S, H = x.shape
    in_maps = prep_inputs(
        x, gate_packed, gate_scale, up_packed, up_scale, down_packed,
        down_scale, norm_w, N_CORES,
    )
    nc = _get_program()
    res = run_bass_kernel_spmd(nc, in_maps, list(range(N_CORES)), trace=_trace)
    out = assemble_output(res.results, B, S, H)
    if _trace:
        kernel.last_results = res
    return out


# revision 25
# speedup vs baseline: 1.0100x; 1.0100x over previous
"""BitNet MLP (nn_BitNetMLP_19421842112750) — TRN2 Bass kernel, 8-core
data-parallel over tokens.  v2: startup/transition/drain overhead removed.

Per core (T=1024 tokens of the 8192 total):
  G = x @ Wg_tern.T ; U = x @ Wu_tern.T          (ternary weights, scales folded)
  r = relu(G)^2 * U                               (= hidden_ref / c, c = gs^2*us)
  hn = r * norm_w ; acc += r^2
  var' = (ones @ acc) / I + eps/c^2               (= (var_ref + eps)/c^2, bcast 128p)
  out = (hn @ Wd_tern.T) * sqrt(ds^2 / var')      (== reference output exactly)

v2 changes vs v1 (1.551 ms):
  - x is pre-cast to bf16 on host and DMA'd straight into the persistent
    xT tile: no f32 staging, no on-device casts (v1 lost 71 us at startup
    because 20 Vector-engine casts preceded the band-0 weight shifts in
    queue order).
  - variance is accumulated on Scalar(square)+Vector(add) engines into an
    SBUF f32 acc tile; one f32 ones-matmul reduces+broadcasts it across
    partitions (v1 spent ~23 us of TensorE on 108 FD-512 variance matmuls
    and serialized phase B behind a PSUM pool handoff).
  - gate/up unpack is widened: one [128, 640B] stage DMA + 4 shifts + 1
    ACT copy per matrix per band (v1: 5 DMAs + 20 shifts + 5 copies) with
    a k-major chunk order absorbed into perm_H.
  - the ragged 256-row down-proj tail is merged into 2 full-K=128 matmuls
    per (ht,t) via half-partition shifts (v1: 4 half-K matmuls that cost
    full FD cycles each; -80 matmuls).
  - down-proj epilogue (rstd mul + store) is emitted per-(ht,t) right
    after that tile's closing matmul, so the final drain is one tile deep
    (v1: 18 us tail), and HBAND=3 + single-buffered PSUM lets ps_var (2
    banks) coexist with the 6 po banks.

Device layouts:
  xTp   [H, T]   bf16 rows: h = pi_H(r)        (per-core x shard, transposed)
  gpTp  [H/4, I] u8   cols: i = pi_I(c)        (same for upTp)
  dpT   [I/4, H] u8   natural (= down_packed.T)
  nw_p  [I]      f32  nw_p[r] = norm_w[pi_I(r)]
  outT  [H, T]   f32  natural h rows (host transposes back)
"""

import sys

sys.path.insert(0, "/opt/trn_rl_repo")
from contextlib import ExitStack

import numpy as np
import ml_dtypes

import concourse.bass as bass
import concourse.tile as tile
from concourse import bacc, mybir

F32 = mybir.dt.float32
BF16 = mybir.dt.bfloat16
U8 = mybir.dt.uint8
U32 = mybir.dt.uint32
SWAR_MASK = 0x03030303
AOT = mybir.AluOpType
ACTF = mybir.ActivationFunctionType
RMS_EPS = 1e-6

N_CORES = 8
FULL_B, FULL_S, FULL_H, FULL_I = 4, 2048, 2560, 6912


# ---------------------------------------------------------------- permutations
def perm_H(n, G=5):
    """SBUF row r = 128*c + p -> original h index, k-major chunk order:
    chunk c holds shift-plane k = c//G of DMA row-group g = c%G."""
    assert n == 512 * G
    r = np.arange(n)
    c, p = r // 128, r % 128
    return 512 * (c % G) + 128 * (p // 32) + 32 * (c // G) + (p % 32)


def perm_I(n):
    """hidden SBUF row r -> original i index. Full 512-groups, then a
    256-tail (two 128-tiles, each split into 64-partition halves)."""
    r = np.arange(n)
    c, p = r // 128, r % 128
    out = 512 * (c // 4) + 128 * (p // 32) + 32 * (c % 4) + (p % 32)
    n_full = (n // 512) * 512
    if n_full != n:
        assert n - n_full == 256, "tail must be exactly 256"
        off = r[n_full:] - n_full
        tile_off, p2 = off // 128, off % 128
        s, q, j = p2 // 64, (p2 % 64) // 32, p2 % 32
        k = 2 * tile_off + s
        out[n_full:] = n_full + 128 * q + 32 * k + j
    return out


# ---------------------------------------------------------------- the program
def build_program(T, H, I, W_I=128, HB=3):
    """Build the single-core Bass program (SPMD-identical across cores)."""
    TC = 512
    G = H // 512  # 5 DMA row-groups per gate/up band
    NH = H // 128  # 20 contraction chunks
    NI = I // 128  # 54 hidden i-tiles
    NT = T // TC  # 2 t-chunks
    NB = I // W_I  # 54 gate/up bands
    IB = I // 4  # 1728 down packed rows
    NC_FULL = IB // 128  # 13 full down C-groups
    C_TAIL = IB % 128  # 64
    NHB = (NH + HB - 1) // HB  # 7 down h-bands
    assert W_I == 128 and T % TC == 0 and H == 512 * G and C_TAIL == 64

    nc = bacc.Bacc("TRN2", target_bir_lowering=False, debug=False)

    xTp = nc.dram_tensor("xTp", [H, T], BF16, kind="ExternalInput").ap()
    gpTp = nc.dram_tensor("gpTp", [H // 4, I], U8, kind="ExternalInput").ap()
    upTp = nc.dram_tensor("upTp", [H // 4, I], U8, kind="ExternalInput").ap()
    dpT = nc.dram_tensor("dpT", [IB, H], U8, kind="ExternalInput").ap()
    nw = nc.dram_tensor("nw_p", [I], F32, kind="ExternalInput").ap()
    eps_in = nc.dram_tensor("eps_in", [128], F32, kind="ExternalInput").ap()
    ds2_in = nc.dram_tensor("ds2_in", [128], F32, kind="ExternalInput").ap()
    # band 0 of gate/up pre-unpacked on host: first matmul needs no on-device
    # unpack (the staging DMA's 640 small descriptors alone cost ~5 us of
    # Sync-queue issue time at startup)
    wg0_in = nc.dram_tensor("wg0", [128, NH * W_I], BF16, kind="ExternalInput").ap()
    wu0_in = nc.dram_tensor("wu0", [128, NH * W_I], BF16, kind="ExternalInput").ap()
    outT = nc.dram_tensor("outT", [H, T], F32, kind="ExternalOutput").ap()

    with tile.TileContext(nc) as tc, ExitStack() as top:
        const = top.enter_context(tc.tile_pool(name="const", bufs=1))
        hn_pool = top.enter_context(tc.tile_pool(name="hn", bufs=1))
        hn_sb = hn_pool.tile([128, NI, T], BF16)
        # phase-B staging pools are created BEFORE the phase-A pools so their
        # SBUF does not recycle A's band buffers: address reuse would add WAR
        # edges making the down-weight prefetch wait for all of phase A
        dstage = top.enter_context(tc.tile_pool(name="dstage", bufs=2))
        dsh = top.enter_context(tc.tile_pool(name="dsh", bufs=2))
        wdp = top.enter_context(tc.tile_pool(name="wdp", bufs=2))
        dtailp = top.enter_context(tc.tile_pool(name="dtailp", bufs=2))
        outp = top.enter_context(tc.tile_pool(name="outp", bufs=2))

        eps_t = const.tile([128, 1], F32)
        ds2_t = const.tile([128, 1], F32)
        nw_sb = const.tile([128, NI], F32)
        ones_f = const.tile([128, 128], F32)
        nc.gpsimd.memset(ones_f[:], 1.0)
        acc = const.tile([128, T], F32)
        nc.gpsimd.memset(acc[:], 0.0)
        rstd_bc = const.tile([128, T], F32)

        # ================= phase A: gate/up matmuls -> hn tiles ==============
        with tc.tile_pool(name="xT", bufs=1) as xT_pool:
            xT = xT_pool.tile([128, NH, T], BF16)
            with (
                tc.tile_pool(name="wband", bufs=2) as wband,
                tc.tile_pool(name="ustage", bufs=2) as ustage,
                tc.tile_pool(name="ush", bufs=2) as ush,
                tc.tile_pool(name="psA", bufs=2, space="PSUM") as psA,
                tc.tile_pool(name="ract", bufs=2) as ract,
                tc.tile_pool(name="sqp", bufs=2) as sqp,
            ):

                def stage_band(band):
                    """DMA + unpack one gate/up band into bf16 weight tiles."""
                    i0 = band * W_I
                    wg_band = wband.tile([128, 4, G, W_I], BF16, tag="wg", name="wg")
                    wu_band = wband.tile([128, 4, G, W_I], BF16, tag="wu", name="wu")
                    for mat_ap, wt in ((gpTp, wg_band), (upTp, wu_band)):
                        st = ustage.tile([128, G, W_I], U8, tag="st", name="st")
                        nc.sync.dma_start(
                            st[:],
                            mat_ap.rearrange("(g p) i -> p g i", p=128)[
                                :, :, i0 : i0 + W_I
                            ],
                        )
                        sh = ush.tile([128, 4, G, W_I], U8, tag="sh", name="sh")
                        stw = st[:].bitcast(U32)
                        for k in range(4):
                            nc.vector.tensor_scalar(
                                sh[:, k].bitcast(U32),
                                stw,
                                6 - 2 * k,
                                SWAR_MASK,
                                AOT.logical_shift_right,
                                AOT.bitwise_and,
                            )
                        # contraction chunk c = k*G + g lives at wt[:, k, g, :]
                        nc.scalar.activation(wt[:], sh[:], ACTF.Copy, bias=-1.0)
                    return wg_band, wu_band

                # band-0 weights first (pre-unpacked on host, 2 fast
                # contiguous DMAs), then x in graduated groups — chunk 0's
                # small group lands early while later groups stream behind
                # the first matmuls; small consts go last
                wg_0 = wband.tile([128, 4, G, W_I], BF16, tag="wg", name="wg")
                wu_0 = wband.tile([128, 4, G, W_I], BF16, tag="wu", name="wu")
                nc.sync.dma_start(
                    wg_0[:], wg0_in.rearrange("p (k g i) -> p k g i", k=4, g=G)
                )
                nc.sync.dma_start(
                    wu_0[:], wu0_in.rearrange("p (k g i) -> p k g i", k=4, g=G)
                )
                staged = (wg_0, wu_0)
                xTp_r = xTp.rearrange("(c p) t -> p c t", p=128)
                for lo, hi in ((0, 2), (2, 4), (4, 8), (8, 12), (12, 16), (16, 20)):
                    nc.sync.dma_start(xT[:, lo:hi, :], xTp_r[:, lo:hi, :])
                nc.sync.dma_start(eps_t[:], eps_in[:, None])
                nc.sync.dma_start(ds2_t[:], ds2_in[:, None])
                nc.sync.dma_start(nw_sb[:], nw.rearrange("(o p) -> p o", p=128))
                for band in range(NB):
                    wg_band, wu_band = staged
                    if band + 1 < NB:
                        staged = stage_band(band + 1)
                    pg = psA.tile([128, T], F32, tag="pg")
                    pu = psA.tile([128, T], F32, tag="pu")

                    for wt, ps in ((wg_band, pg), (wu_band, pu)):
                        for c in range(NH):
                            for t in range(NT):
                                tsl = slice(TC * t, TC * (t + 1))
                                nc.tensor.matmul(
                                    ps[:, tsl],
                                    wt[:, c // G, c % G, :],
                                    xT[:, c, tsl],
                                    start=(c == 0),
                                    stop=(c == NH - 1),
                                )
                    for t in range(NT):
                        tsl = slice(TC * t, TC * (t + 1))
                        r = ract.tile([128, TC], BF16, tag="r")
                        nc.scalar.activation(r[:], pg[:, tsl], ACTF.Relu)
                        nc.vector.tensor_mul(r[:], r[:], r[:])
                        nc.vector.tensor_mul(r[:], r[:], pu[:, tsl])
                        sq = sqp.tile([128, TC], BF16, tag="sq")
                        nc.scalar.activation(sq[:], r[:], ACTF.Square)
                        nc.vector.tensor_scalar(
                            hn_sb[:, band, tsl], r[:], nw_sb[:, band : band + 1],
                            None, AOT.mult,
                        )
                        nc.vector.tensor_add(acc[:, tsl], acc[:, tsl], sq[:])

        # ====== phase A2 + B: variance reduce/bcast overlapped with down =====
        with (
            tc.tile_pool(name="psV", bufs=1, space="PSUM") as psV,
            tc.tile_pool(name="psB", bufs=1, space="PSUM") as psB,
        ):
            ps_var = psV.tile([128, T], F32)

            def emit_variance(step):
                # variance chain sliced across the first down C-groups: the
                # ones-matmul reduces acc over partitions AND broadcasts to
                # all 128 output partitions; the 2x ~3.3 us reciprocal halves
                # are spread out so they don't head-of-line-block the Vector
                # queue in front of the down-weight shifts
                if step == 0:
                    for t in range(NT):
                        tsl = slice(TC * t, TC * (t + 1))
                        nc.tensor.matmul(
                            ps_var[:, tsl], ones_f[:], acc[:, tsl],
                            start=True, stop=True,
                        )
                    nc.scalar.activation(
                        acc[:], ps_var[:], ACTF.Identity, bias=eps_t[:], scale=1.0 / I
                    )
                elif step in (1, 2):
                    tsl = slice(TC * (step - 1), TC * step)
                    nc.vector.reciprocal(acc[:, tsl], acc[:, tsl])
                elif step == 3:
                    nc.scalar.activation(rstd_bc[:], acc[:], ACTF.Sqrt, scale=ds2_t[:])

            for hb in range(NHB):
                hbs = min(HB, NH - HB * hb)
                h0 = 128 * HB * hb
                W = 128 * hbs
                po = [
                    [
                        psB.tile([128, TC], F32, tag=f"po_{ht}_{t}", name=f"po_{ht}_{t}")
                        for t in range(NT)
                    ]
                    for ht in range(hbs)
                ]
                for C in range(NC_FULL):
                    st = dstage.tile([128, W], U8, tag="dst")
                    nc.sync.dma_start(
                        st[:], dpT[128 * C : 128 * (C + 1), h0 : h0 + W]
                    )
                    sh4 = dsh.tile([128, 4, W], U8, tag="dsh")
                    stw = st[:].bitcast(U32)
                    for k in range(4):
                        nc.vector.tensor_scalar(
                            sh4[:, k].bitcast(U32),
                            stw,
                            6 - 2 * k,
                            SWAR_MASK,
                            AOT.logical_shift_right,
                            AOT.bitwise_and,
                        )
                    wdt = wdp.tile([128, 4, W], BF16, tag="wd")
                    nc.scalar.activation(wdt[:], sh4[:], ACTF.Copy, bias=-1.0)
                    for k in range(4):
                        for ht in range(hbs):
                            hsl = slice(128 * ht, 128 * (ht + 1))
                            for t in range(NT):
                                tsl = slice(TC * t, TC * (t + 1))
                                nc.tensor.matmul(
                                    po[ht][t][:, :],
                                    wdt[:, k, hsl],
                                    hn_sb[:, 4 * C + k, tsl],
                                    start=(C == 0 and k == 0),
                                    stop=False,
                                )
                    if hb == 0 and C < 4:
                        emit_variance(C)
                # merged 256-row tail: the 64 dpT tail rows are replicated
                # into both partition halves; half-partition shifts build
                # plane-pairs (k0,k1) / (k2,k3) matching hn tiles NI-2/NI-1
                stt = dtailp.tile([128, W], U8, tag="dtail", name="stt")
                src = dpT[IB - C_TAIL : IB, h0 : h0 + W]
                nc.sync.dma_start(stt[:C_TAIL], src)
                nc.sync.dma_start(stt[C_TAIL:], src)
                sht = dtailp.tile([128, 2, W], U8, tag="dshT", name="sht")
                sttw = stt[:].bitcast(U32)
                for j in range(2):
                    nc.vector.tensor_scalar(
                        sht[:C_TAIL, j].bitcast(U32), sttw[:C_TAIL],
                        6 - 4 * j, SWAR_MASK,
                        AOT.logical_shift_right, AOT.bitwise_and,
                    )
                    nc.vector.tensor_scalar(
                        sht[C_TAIL:, j].bitcast(U32), sttw[C_TAIL:],
                        4 - 4 * j, SWAR_MASK,
                        AOT.logical_shift_right, AOT.bitwise_and,
                    )
                wtl = dtailp.tile([128, 2, W], BF16, tag="wdT", name="wtl")
                nc.scalar.activation(wtl[:], sht[:], ACTF.Copy, bias=-1.0)
                # close each po tile and emit its epilogue immediately
                for ht in range(hbs):
                    hsl = slice(128 * ht, 128 * (ht + 1))
                    for t in range(NT):
                        tsl = slice(TC * t, TC * (t + 1))
                        nc.tensor.matmul(
                            po[ht][t][:, :], wtl[:, 0, hsl],
                            hn_sb[:, NI - 2, tsl], start=False, stop=False,
                        )
                        nc.tensor.matmul(
                            po[ht][t][:, :], wtl[:, 1, hsl],
                            hn_sb[:, NI - 1, tsl], start=False, stop=True,
                        )
                        ot = outp.tile([128, TC], F32, tag="ot")
                        nc.vector.tensor_mul(ot[:], po[ht][t][:], rstd_bc[:, tsl])
                        nc.sync.dma_start(
                            outT[h0 + 128 * ht : h0 + 128 * (ht + 1), tsl], ot[:]
                        )

    nc.compile()
    return nc


# ------------------------------------------------------------- host-side prep
def _unpack_rows(packed, K):
    """SIMD block-interleaved 2-bit rows -> ternary float rows [M, K]."""
    M = packed.shape[0]
    b = packed.astype(np.int16).reshape(M, K // 128, 32)
    w = np.stack([(b >> 6) & 3, (b >> 4) & 3, (b >> 2) & 3, b & 3], axis=2)
    return w.reshape(M, K).astype(np.float32) - 1.0


def _band0_device_layout(packed, piH, piI, H):
    """Pre-unpacked band-0 weights in the device tile layout [128, NH*128]."""
    w = _unpack_rows(packed[piI[:128]], H)[:, piH]  # [128 ji, H r]
    return np.ascontiguousarray(
        w.T.reshape(H // 128, 128, 128).transpose(1, 0, 2).reshape(128, H)
    ).astype(ml_dtypes.bfloat16)


def prep_inputs(x, gate_packed, gate_scale, up_packed, up_scale, down_packed,
                down_scale, norm_w, n_cores):
    """Full inputs -> per-core in_maps in device layout (slicing + relayout)."""
    B, S, H = x.shape
    I = norm_w.shape[0]
    T_full = B * S
    T = T_full // n_cores
    piH = perm_H(H)
    piI = perm_I(I)

    gpTp = np.ascontiguousarray(gate_packed[piI].T)  # [H/4, I]
    upTp = np.ascontiguousarray(up_packed[piI].T)
    dpT = np.ascontiguousarray(down_packed.T)  # [I/4, H]
    nw_p = np.ascontiguousarray(norm_w[piI]).astype(np.float32)
    xf = x.reshape(T_full, H)

    gs_v = float(np.asarray(gate_scale).reshape(-1)[0])
    us_v = float(np.asarray(up_scale).reshape(-1)[0])
    ds_v = float(np.asarray(down_scale).reshape(-1)[0])
    c = gs_v * gs_v * us_v
    eps_p = np.full([128], RMS_EPS / (c * c), np.float32)
    ds2 = np.full([128], ds_v * ds_v, np.float32)
    wg0 = _band0_device_layout(gate_packed, piH, piI, H)
    wu0 = _band0_device_layout(up_packed, piH, piI, H)

    in_maps = []
    for cid in range(n_cores):
        xs = xf[cid * T : (cid + 1) * T]
        xTp = np.ascontiguousarray(xs.T[piH]).astype(ml_dtypes.bfloat16)  # [H, T]
        in_maps.append(
            {
                "xTp": xTp,
                "gpTp": gpTp,
                "upTp": upTp,
                "dpT": dpT,
                "nw_p": nw_p,
                "eps_in": eps_p,
                "ds2_in": ds2,
                "wg0": wg0,
                "wu0": wu0,
            }
        )
    return in_maps


def assemble_output(results, B, S, H):
    """Per-core outT [H, T] -> full [B, S, H]."""
    outs = [np.asarray(r["outT"]).T for r in results]  # each [T, H]
    return np.ascontiguousarray(np.concatenate(outs, axis=0).reshape(B, S, H))


# ---------------------------------------------------------------- entry point
_CACHED = {}


def _get_program():
    if "nc" not in _CACHED:
        T = FULL_B * FULL_S // N_CORES
        _CACHED["nc"] = build_program(T, FULL_H, FULL_I)
    return _CACHED["nc"]


def kernel(x, gate_packed, gate_scale, up_packed, up_scale, down_packed,
           down_scale, norm_w, _trace=False):
    from concourse.bass_utils import run_bass_kernel_spmd

    x = np.asarray(x, np.float32)
    gate_packed = np.asarray(gate_packed, np.uint8)
    up_packed = np.asarray(up_packed, np.uint8)
    down_packed = np.asarray(down_packed, np.uint8)
    norm_w = np.asarray(norm_w, np.float32)

    B, S, H = x.shape
    in_maps = prep_inputs(
        x, gate_packed, gate_scale, up_packed, up_scale, down_packed,
        down_scale, norm_w, N_CORES,
    )
    nc = _get_program()
    res = run_bass_kernel_spmd(nc, in_maps, list(range(N_CORES)), trace=_trace)
    out = assemble_output(res.results, B, S, H)
    if _trace:
        kernel.last_results = res
    return out
